# revision 1
# baseline (speedup 1.0000x reference)
# Causal self-attention (B=8, T=1024, C=1024, H=16, D=64) on 8 trn2 NeuronCores.
# Sharding: data-parallel over batch — core i computes batch element i entirely
# (weights replicated, no collectives).
#
# Host-side prep (inside kernel(), per core): x pre-transposed to xT [C,T] and
# cast bf16; W_qkv/W_out pre-packed bf16 into [p, cc, j] chunk layout so every
# DMA is contiguous and no PE transposes are needed on device.
#
# Per-core pipeline (all matmuls bf16, fp32 PSUM accumulation):
#   A: v[t,j] (per-head with ones column, width 65, for the softmax denom),
#      qk groups for head-pairs 0,1 (half 0).
#   B: for it (query 512-tile) 0,1; for head pair hp: per key chunk jc:
#      scores sT[j,i] (K=64 pair, row bases 0/64, causally trimmed),
#      exp on ACT (scale=1/8) -> pT bf16 SBUF,
#      post-exp causal mask of the diag 128-block via gpsimd affine_select,
#      attn@v (M=65; row 64 = l = sum_j p) with a 2-unit software-pipeline lag.
#      Normalize: DVE reciprocal of l row, gpsimd partition_broadcast,
#      DVE multiply -> oT bf16.
#      Remaining qk groups + out-proj token tiles 0..3 are interleaved as PE
#      filler so the tensor engine never idles (keeps HAM at full clock).
#   C: out-proj token tiles 4..7 -> DMA to DRAM per (ti, half).

from collections import deque
from contextlib import ExitStack

import numpy as np
import ml_dtypes

import concourse.bass as bass
import concourse.bacc as bacc
import concourse.mybir as mybir
import concourse.tile as tile
from concourse import bass_utils

FP32 = mybir.dt.float32
BF16 = mybir.dt.bfloat16

B, T, C = 8, 1024, 1024
H, D = 16, 64
N_CORES = 8
CCH = C // 128   # 8 contraction chunks of 128
TCH = T // 128   # 8 token chunks of 128


def build_nc():
    nc = bacc.Bacc("TRN2", debug=False, num_devices=N_CORES)

    xT_d = nc.dram_tensor("xT_b", [C, T], BF16, kind="ExternalInput").ap()
    wv_d = nc.dram_tensor("wv", [2, 128, CCH, 512], BF16, kind="ExternalInput").ap()
    wq_d = nc.dram_tensor("wq", [16, 128, CCH, 128], BF16, kind="ExternalInput").ap()
    wo_d = nc.dram_tensor("wo", [128, CCH, C], BF16, kind="ExternalInput").ap()
    bqT_d = nc.dram_tensor("bqT", [128, 16], FP32, kind="ExternalInput").ap()
    bqv_d = nc.dram_tensor("bqv", [1, C], FP32, kind="ExternalInput").ap()
    bo_d = nc.dram_tensor("bo", [1, C], FP32, kind="ExternalInput").ap()
    out_d = nc.dram_tensor("out_b", [T, C], FP32, kind="ExternalOutput").ap()

    ADD = mybir.AluOpType.add
    MULT = mybir.AluOpType.mult

    with tile.TileContext(nc) as tc, ExitStack() as ctx:
        consts = ctx.enter_context(tc.tile_pool(name="consts", bufs=1))
        wpool = ctx.enter_context(tc.tile_pool(name="weights", bufs=1))
        apool = ctx.enter_context(tc.tile_pool(name="acts", bufs=1))
        ppool = ctx.enter_context(tc.tile_pool(name="ppool", bufs=4))
        lpool = ctx.enter_context(tc.tile_pool(name="lpool", bufs=2))
        rbpool = ctx.enter_context(tc.tile_pool(name="rbpool", bufs=2))
        outs = ctx.enter_context(tc.tile_pool(name="outs", bufs=3))
        # PSUM (8 banks): P1 2x[128,512]=2, P2 2x[128,2,512]=4, P3 2x[65,512]=2
        P1 = ctx.enter_context(tc.tile_pool(name="P1", bufs=2, space="PSUM"))
        P2 = ctx.enter_context(tc.tile_pool(name="P2", bufs=2, space="PSUM"))
        P3 = ctx.enter_context(tc.tile_pool(name="P3", bufs=1, space="PSUM"))

        # ---- tiny const DMAs first ----
        bqv_sb = consts.tile([1, C], FP32, tag="bqv")
        nc.gpsimd.dma_start(out=bqv_sb, in_=bqv_d)
        bo_sb = consts.tile([1, C], FP32, tag="bo")
        nc.gpsimd.dma_start(out=bo_sb, in_=bo_d)
        bqT = consts.tile([128, 16], FP32, tag="bqT")
        nc.gpsimd.dma_start(out=bqT, in_=bqT_d)

        # ---- persistent activations ----
        xT = [apool.tile([128, T], BF16, tag=f"xT{cc}", name=f"xT{cc}")
              for cc in range(CCH)]
        qkT = [apool.tile([128, T], BF16, tag=f"qkT{jt}", name=f"qkT{jt}")
               for jt in range(16)]
        vp = [apool.tile([128, H * (D + 1)], BF16, tag=f"vp{t_}", name=f"vp{t_}")
              for t_ in range(TCH)]
        oT = [apool.tile([128, T], BF16, tag=f"oT{hc}", name=f"oT{hc}")
              for hc in range(CCH)]

        # ---- x (pre-transposed on host) on two DMA queues ----
        for cc in range(4):
            nc.gpsimd.dma_start(out=xT[cc], in_=xT_d[cc * 128:(cc + 1) * 128, :])
        for cc in range(4, CCH):
            nc.scalar.dma_start(out=xT[cc], in_=xT_d[cc * 128:(cc + 1) * 128, :])

        # ---- weights: contiguous DMAs, consumption order, two queues ----
        wv_sb = [wpool.tile([128, CCH, 512], BF16, tag=f"wv{j}", name=f"wv{j}")
                 for j in range(2)]
        wq_sb = [wpool.tile([128, CCH, 128], BF16, tag=f"wq{jt}", name=f"wq{jt}")
                 for jt in range(16)]
        wo_sb = wpool.tile([128, CCH, C], BF16, tag="wo", name="wo")
        for jt in (0, 8):
            nc.sync.dma_start(out=wq_sb[jt], in_=wq_d[jt])
        nc.sync.dma_start(out=wv_sb[0], in_=wv_d[0])
        for jt in (1, 9):
            nc.sync.dma_start(out=wq_sb[jt], in_=wq_d[jt])
        nc.sync.dma_start(out=wv_sb[1], in_=wv_d[1])
        for jt in (2, 10, 3, 11):
            nc.sync.dma_start(out=wq_sb[jt], in_=wq_d[jt])
        for jt in (4, 12, 5, 13, 6, 14, 7, 15):
            nc.scalar.dma_start(out=wq_sb[jt], in_=wq_d[jt])
        nc.scalar.dma_start(out=wo_sb, in_=wo_d)

        # ---- bias broadcasts (gpsimd; no PSUM, no PE) ----
        bvb = consts.tile([128, C], FP32, tag="bvb")
        nc.gpsimd.partition_broadcast(out_ap=bvb, in_ap=bqv_sb)
        bob = consts.tile([128, C], FP32, tag="bob")
        nc.gpsimd.partition_broadcast(out_ap=bob, in_ap=bo_sb)

        # ones columns of vp (denominator trick), once
        for ti in range(TCH):
            vcol = vp[ti].rearrange("p (h d) -> p h d", h=H)
            nc.vector.memset(vcol[:, :, D:D + 1], 1.0)

        # ---- phase A: v projection (jvt-outer: wv1 lands later), qk prefill --
        def vproj(ti, jvt):
            vcol = vp[ti].rearrange("p (h d) -> p h d", h=H)
            ps = P1.tile([128, 512], FP32, tag="p1", name="psv")
            for cc in range(CCH):
                nc.tensor.matmul(
                    out=ps,
                    lhsT=xT[cc][:, ti * 128:(ti + 1) * 128],
                    rhs=wv_sb[jvt][:, cc, :],
                    start=(cc == 0), stop=(cc == CCH - 1))
            nc.vector.tensor_tensor(
                out=vcol[:, jvt * 8:(jvt + 1) * 8, 0:D],
                in0=ps.rearrange("p (h d) -> p h d", h=8),
                in1=bvb[:, jvt * 512:(jvt + 1) * 512].rearrange(
                    "p (h d) -> p h d", h=8),
                op=ADD)

        def qk_group(jt, half):
            sl = slice(half * 512, (half + 1) * 512)
            ps = P1.tile([128, 512], FP32, tag="p1", name="psqk")
            for cc in range(CCH):
                nc.tensor.matmul(
                    out=ps,
                    lhsT=wq_sb[jt][:, cc, :],
                    rhs=xT[cc][:, sl],
                    start=(cc == 0), stop=(cc == CCH - 1))
            nc.vector.tensor_scalar_add(
                out=qkT[jt][:, sl], in0=ps, scalar1=bqT[:, jt:jt + 1])

        def outproj(ti, half):
            sl = slice(half * 512, (half + 1) * 512)
            ps = P1.tile([128, 512], FP32, tag="p1", name="pso")
            for hc in range(CCH):
                nc.tensor.matmul(
                    out=ps,
                    lhsT=oT[hc][:, ti * 128:(ti + 1) * 128],
                    rhs=wo_sb[:, hc, sl],
                    start=(hc == 0), stop=(hc == CCH - 1))
            ot = outs.tile([128, 512], FP32, tag="ot", name="ot")
            nc.vector.tensor_tensor(out=ot, in0=ps, in1=bob[:, sl], op=ADD)
            nc.sync.dma_start(
                out=out_d[ti * 128:(ti + 1) * 128, sl], in_=ot)

        for jt in (0, 8):
            qk_group(jt, 0)
        for ti in range(TCH):
            vproj(ti, 0)
        for jt in (1, 9):
            qk_group(jt, 0)
        for ti in range(TCH):
            vproj(ti, 1)

        # ---- phase B: attention, it-outer, with PE filler interleave ----
        def make_attnv(hp, jc, njc, pT, s0, po2):
            def go():
                if jc == 0:
                    # allocate lazily at emission time so the ring-slot reuse
                    # dependency sees the previous pair's (lagged) norm reads
                    po2[0] = P3.tile([65, 512], FP32, tag="po0", name="po0")
                    po2[1] = P3.tile([65, 512], FP32, tag="po1", name="po1")
                for hx, h in enumerate((2 * hp, 2 * hp + 1)):
                    hsl = slice(h * (D + 1), h * (D + 1) + D + 1)
                    nc.tensor.matmul(
                        out=po2[hx][0:65, s0:512],
                        lhsT=vp[jc][:, hsl],
                        rhs=pT[:, hx, s0:512],
                        start=(jc == 0), stop=(jc == njc - 1),
                        skip_group_check=True)
            return go

        def make_norm(hp, it, po2):
            def go():
                for hx in range(2):
                    l_sb = lpool.tile([1, 512], FP32, tag="l", name="l")
                    # custom-DVE ops misread partition-offset inputs; extract
                    # the l row with a plain copy first
                    nc.vector.tensor_copy(out=l_sb, in_=po2[hx][64:65, :])
                    rb1 = lpool.tile([1, 512], FP32, tag="rb1", name="rb1")
                    nc.vector.reciprocal_approx_fast(out=rb1, in_=l_sb)
                    rb = rbpool.tile([64, 512], FP32, tag="rb", name="rb")
                    nc.gpsimd.partition_broadcast(out_ap=rb, in_ap=rb1)
                    nc.vector.tensor_tensor(
                        out=oT[hp][hx * 64:(hx + 1) * 64,
                                   it * 512:(it + 1) * 512],
                        in0=po2[hx][0:64, :], in1=rb, op=MULT)
            return go

        # filler schedules: {unit_index: [closures]}
        f_b1 = {}
        b1q = [(2, 0), (10, 0), (3, 0), (11, 0), (4, 0), (12, 0), (5, 0),
               (13, 0), (6, 0), (14, 0), (7, 0), (15, 0), (0, 1), (8, 1),
               (1, 1), (9, 1)]
        for u, g in enumerate(b1q):
            f_b1.setdefault(u, []).append(g)
        f_b2 = {}
        b2q = [(2, 1), (10, 1), (3, 1), (11, 1), (4, 1), (12, 1), (5, 1),
               (13, 1), (6, 1), (14, 1), (7, 1), (15, 1)]
        for u, g in enumerate(b2q):
            f_b2.setdefault(u, []).append(g)
        b2o = [(0, 0), (0, 1), (1, 0), (1, 1), (2, 0), (2, 1), (3, 0), (3, 1)]
        for i, o in enumerate(b2o):
            f_b2.setdefault(24 + 3 * i, []).append(("OUT", o))

        LAG = 2
        pending = deque()

        def flush(n_keep):
            while len(pending) > n_keep:
                pending.popleft()()

        for it in (0, 1):
            fill = f_b1 if it == 0 else f_b2
            njc = 4 * (it + 1)
            u = 0
            for hp in range(8):
                po2 = [None, None]
                for jc in range(njc):
                    s0 = max(0, jc * 128 - it * 512)
                    ps = P2.tile([128, 2, 512], FP32, tag="ps", name="ps")
                    pT = ppool.tile([128, 2, 512], BF16, tag="pT", name="pT")
                    for hx in range(2):
                        prow = slice(hx * 64, hx * 64 + 64)
                        nc.tensor.matmul(
                            out=ps[:, hx, s0:512],
                            lhsT=qkT[8 + hp][prow, jc * 128:(jc + 1) * 128],
                            rhs=qkT[hp][prow, it * 512 + s0:(it + 1) * 512],
                            start=True, stop=True)
                    nc.scalar.activation(
                        out=pT[:, :, s0:512], in_=ps[:, :, s0:512],
                        func=mybir.ActivationFunctionType.Exp, scale=0.125)
                    if jc >= it * 4:  # diagonal block: post-exp causal zeroing
                        nc.gpsimd.affine_select(
                            out=pT[:, :, s0:s0 + 128],
                            in_=pT[:, :, s0:s0 + 128],
                            compare_op=mybir.AluOpType.is_ge, fill=0.0,
                            base=0, channel_multiplier=-1,
                            pattern=[[0, 2], [1, 128]])
                    for f in fill.get(u, ()):
                        if isinstance(f, tuple) and f and f[0] == "OUT":
                            outproj(*f[1])
                        else:
                            qk_group(*f)
                    pending.append(make_attnv(hp, jc, njc, pT, s0, po2))
                    flush(LAG)
                    u += 1
                pending.append(make_norm(hp, it, po2))
            flush(0)

        # ---- phase C: out projection, token tiles 4..7 ----
        for ti in range(4, TCH):
            for half in range(2):
                outproj(ti, half)

    nc.compile()
    nc.finalize()
    return nc


_CACHE = {}


def _bf16(a):
    return np.ascontiguousarray(np.asarray(a, np.float32)).astype(
        ml_dtypes.bfloat16)


def make_in_maps(x, W_qkv, b_qkv, W_out, b_out):
    W_qkv = np.asarray(W_qkv, np.float32)
    W_out = np.asarray(W_out, np.float32)
    b_qkv = np.asarray(b_qkv, np.float32).reshape(-1)
    b_out = np.asarray(b_out, np.float32).reshape(1, -1)
    x = np.asarray(x, np.float32)

    wv = np.stack([
        _bf16(W_qkv[:, 2 * C + j * 512:2 * C + (j + 1) * 512]
              .reshape(CCH, 128, 512).transpose(1, 0, 2))
        for j in range(2)])
    wq = np.stack([
        _bf16(W_qkv[:, jt * 128:(jt + 1) * 128]
              .reshape(CCH, 128, 128).transpose(1, 0, 2))
        for jt in range(16)])
    wo = _bf16(W_out.reshape(CCH, 128, C).transpose(1, 0, 2))
    bqT = np.ascontiguousarray(b_qkv[:2 * C].reshape(16, 128).T)
    bqv = np.ascontiguousarray(b_qkv[2 * C:].reshape(1, C))

    shared = {
        "wv": wv, "wq": wq, "wo": wo,
        "bqT": bqT, "bqv": bqv, "bo": np.ascontiguousarray(b_out),
    }
    return [dict(shared, xT_b=_bf16(x[i].T)) for i in range(N_CORES)]


def kernel(x, W_qkv, b_qkv, W_out, b_out):
    if "nc" not in _CACHE:
        _CACHE["nc"] = build_nc()
    nc = _CACHE["nc"]
    in_maps = make_in_maps(x, W_qkv, b_qkv, W_out, b_out)
    res = bass_utils.run_bass_kernel_spmd(nc, in_maps, core_ids=list(range(N_CORES)))
    return np.stack([r["out_b"] for r in res.results]).astype(np.float32)



# revision 8
# speedup vs baseline: 1.0534x; 1.0534x over previous
# Causal self-attention (B=8, T=1024, C=1024, H=16, D=64) on 8 trn2 NeuronCores.
# Sharding: data-parallel over batch — core i computes batch element i entirely
# (weights replicated, no collectives).
#
# v2: Q/K projections run as fp8e4 DoubleRow matmuls (two 128-deep contraction
# chunks per pass). Weights are pre-scaled by 32 on the host so their +-1/32
# range maps into fp8 normal range; the DVE consumer rescales by 1/32 while
# adding the bias (tensor_scalar mult+add). Softmax renormalization absorbs the
# q/k quantization error (measured ~0.8% end-to-end). The v / attn / out-proj
# paths stay bf16 — fp8 there costs ~4% end-to-end which exceeds tolerance.
#
# Per-core pipeline (fp32 PSUM accumulation everywhere):
#   A: qk groups for head-pairs 0,1 (fp8 DR), v[t,j] per-head with a leading
#      denominator column (width 65, ones-first so the softmax-l row lands at
#      PSUM partition 0 where the custom-DVE reciprocal can read it directly).
#   B: for it (query 512-tile) 0,1; head pair hp; key chunk jc:
#      scores sT[j,i] (K=64 pair, row bases 0/64 -> auto row-tiled, concurrent),
#      exp on ACT (scale=1/8) -> pT bf16, gpsimd affine_select causal mask on
#      the diagonal 128-block, attn@v (M=65; row 0 = l = sum_j p) with a 2-unit
#      software-pipeline lag. Normalize: DVE reciprocal of l row (direct from
#      PSUM), gpsimd partition_broadcast, DVE multiply -> oT bf16.
#      Remaining qk groups + out-proj token tiles 0..3 interleave as PE filler.
#   C: out-proj token tiles 4..7 -> DMA to DRAM per (ti, half).

from collections import deque
from contextlib import ExitStack

import numpy as np
import ml_dtypes

import concourse.bass as bass
import concourse.bacc as bacc
import concourse.mybir as mybir
import concourse.tile as tile
from concourse import bass_utils

FP32 = mybir.dt.float32
BF16 = mybir.dt.bfloat16
FP8 = mybir.dt.float8e4

B, T, C = 8, 1024, 1024
H, D = 16, 64
N_CORES = 8
CCH = C // 128   # 8 contraction chunks of 128
TCH = T // 128   # 8 token chunks of 128
NPAIR = 4        # 4 DoubleRow pairs of contraction chunks
WS = 32.0        # host-side fp8 weight scale (power of two)
DR = mybir.MatmulPerfMode.DoubleRow


def build_nc():
    nc = bacc.Bacc("TRN2", debug=False, num_devices=N_CORES)

    x8_d = nc.dram_tensor("x8_b", [NPAIR, 128, 2, T], FP8, kind="ExternalInput").ap()
    xT_d = nc.dram_tensor("xT_b", [C, T], BF16, kind="ExternalInput").ap()
    wq8_d = nc.dram_tensor("wq8", [16, 128, NPAIR, 2, 128], FP8,
                           kind="ExternalInput").ap()
    wv_d = nc.dram_tensor("wv", [2, 128, CCH, 512], BF16, kind="ExternalInput").ap()
    wo_d = nc.dram_tensor("wo", [2, 128, CCH, 512], BF16, kind="ExternalInput").ap()
    bqT_d = nc.dram_tensor("bqT", [128, 16], FP32, kind="ExternalInput").ap()
    bqv_d = nc.dram_tensor("bqv", [1, C], FP32, kind="ExternalInput").ap()
    bo_d = nc.dram_tensor("bo", [1, C], FP32, kind="ExternalInput").ap()
    out_d = nc.dram_tensor("out_b", [T, C], FP32, kind="ExternalOutput").ap()

    ADD = mybir.AluOpType.add
    MULT = mybir.AluOpType.mult

    with tile.TileContext(nc) as tc, ExitStack() as ctx:
        consts = ctx.enter_context(tc.tile_pool(name="consts", bufs=1))
        wpool = ctx.enter_context(tc.tile_pool(name="weights", bufs=1))
        apool = ctx.enter_context(tc.tile_pool(name="acts", bufs=1))
        ppool = ctx.enter_context(tc.tile_pool(name="ppool", bufs=4))
        lpool = ctx.enter_context(tc.tile_pool(name="lpool", bufs=2))
        rbpool = ctx.enter_context(tc.tile_pool(name="rbpool", bufs=2))
        outs = ctx.enter_context(tc.tile_pool(name="outs", bufs=3))
        # PSUM (8 banks): P1 2x[128,512]=2, P2 2x[128,2,512]=4, P3 2x[65,512]=2
        P1 = ctx.enter_context(tc.tile_pool(name="P1", bufs=2, space="PSUM"))
        P2 = ctx.enter_context(tc.tile_pool(name="P2", bufs=2, space="PSUM"))
        P3 = ctx.enter_context(tc.tile_pool(name="P3", bufs=1, space="PSUM"))

        # ---- tiny const DMAs first ----
        bqv_sb = consts.tile([1, C], FP32, tag="bqv")
        nc.gpsimd.dma_start(out=bqv_sb, in_=bqv_d)
        bo_sb = consts.tile([1, C], FP32, tag="bo")
        nc.gpsimd.dma_start(out=bo_sb, in_=bo_d)
        bqT = consts.tile([128, 16], FP32, tag="bqT")
        nc.gpsimd.dma_start(out=bqT, in_=bqT_d)

        # ---- persistent activations ----
        x8 = [apool.tile([128, 2, T], FP8, tag=f"x8{p}", name=f"x8{p}")
              for p in range(NPAIR)]
        xT = [apool.tile([128, T], BF16, tag=f"xT{cc}", name=f"xT{cc}")
              for cc in range(CCH)]
        qkT = [apool.tile([128, T], BF16, tag=f"qkT{jt}", name=f"qkT{jt}")
               for jt in range(16)]
        vp = [apool.tile([128, H * (D + 1)], BF16, tag=f"vp{t_}", name=f"vp{t_}")
              for t_ in range(TCH)]
        oT = [apool.tile([128, T], BF16, tag=f"oT{hc}", name=f"oT{hc}")
              for hc in range(CCH)]

        # ---- DMAs (3 issue queues: sync / scalar / gpsimd).  x8 + wq8[0,8]
        # first (unblocks phase A), then xT/wv (v path), then the rest in
        # consumption order; wo (needed ~40us in) last. ----
        wq8_sb = [wpool.tile([128, NPAIR, 2, 128], FP8, tag=f"wq{jt}",
                             name=f"wq{jt}") for jt in range(16)]
        # wv split per contraction chunk so vproj's cc-loop can start early
        wv_sb = [[wpool.tile([128, 512], BF16, tag=f"wv{j}_{cc}",
                             name=f"wv{j}_{cc}") for cc in range(CCH)]
                 for j in range(2)]
        wo_sb = [wpool.tile([128, CCH, 512], BF16, tag=f"wo{h}", name=f"wo{h}")
                 for h in range(2)]

        nc.sync.dma_start(out=x8[0], in_=x8_d[0])
        nc.scalar.dma_start(out=x8[1], in_=x8_d[1])
        nc.gpsimd.dma_start(out=x8[2], in_=x8_d[2])
        nc.sync.dma_start(out=x8[3], in_=x8_d[3])
        nc.scalar.dma_start(out=wq8_sb[0], in_=wq8_d[0])
        nc.gpsimd.dma_start(out=wq8_sb[8], in_=wq8_d[8])
        nc.sync.dma_start(out=wq8_sb[1], in_=wq8_d[1])
        nc.scalar.dma_start(out=wq8_sb[9], in_=wq8_d[9])
        nc.gpsimd.dma_start(out=wq8_sb[2], in_=wq8_d[2])
        nc.sync.dma_start(out=wq8_sb[10], in_=wq8_d[10])
        # v path: xT chunks + wv chunks interleaved across queues
        for cc in range(CCH):
            q = (nc.sync, nc.scalar, nc.gpsimd)[cc % 3]
            q.dma_start(out=xT[cc], in_=xT_d[cc * 128:(cc + 1) * 128, :])
        for cc in range(CCH):
            q = (nc.gpsimd, nc.sync, nc.scalar)[cc % 3]
            q.dma_start(out=wv_sb[0][cc], in_=wv_d[0, :, cc, :])
        for cc in range(CCH):
            q = (nc.scalar, nc.gpsimd, nc.sync)[cc % 3]
            q.dma_start(out=wv_sb[1][cc], in_=wv_d[1, :, cc, :])
        # remaining qk weights (fillers), then wo
        for i, jt in enumerate((3, 11, 4, 12, 5, 13, 6, 14, 7, 15)):
            q = (nc.sync, nc.scalar, nc.gpsimd)[i % 3]
            q.dma_start(out=wq8_sb[jt], in_=wq8_d[jt])
        nc.scalar.dma_start(out=wo_sb[0], in_=wo_d[0])
        nc.gpsimd.dma_start(out=wo_sb[1], in_=wo_d[1])

        # ---- bias broadcasts (gpsimd; no PSUM, no PE) ----
        bvb = consts.tile([128, C], FP32, tag="bvb")
        nc.gpsimd.partition_broadcast(out_ap=bvb, in_ap=bqv_sb)
        bob = consts.tile([128, C], FP32, tag="bob")
        nc.gpsimd.partition_broadcast(out_ap=bob, in_ap=bo_sb)

        # denominator (ones) columns of vp, once.  Ones sit LAST within each
        # head's 65-wide block: PSUM partition access must be 32-aligned, so
        # the norm-mult must read po[0:64] (offset 0) and l lives at row 64.
        for ti in range(TCH):
            vcol = vp[ti].rearrange("p (h d) -> p h d", h=H)
            nc.vector.memset(vcol[:, :, D:D + 1], 1.0)

        # ---- building blocks ----
        def qk_group(jt, half):
            sl = slice(half * 512, (half + 1) * 512)
            ps = P1.tile([128, 512], FP32, tag="p1", name="psqk")
            for p in range(NPAIR):
                nc.tensor.matmul(
                    out=ps,
                    lhsT=wq8_sb[jt][:, p],
                    rhs=x8[p][:, :, sl],
                    start=(p == 0), stop=(p == NPAIR - 1),
                    perf_mode=DR)
            nc.vector.tensor_scalar(
                out=qkT[jt][:, sl], in0=ps,
                scalar1=1.0 / WS, scalar2=bqT[:, jt:jt + 1],
                op0=MULT, op1=ADD)

        def vproj(ti, jvt):
            vcol = vp[ti].rearrange("p (h d) -> p h d", h=H)
            ps = P1.tile([128, 512], FP32, tag="p1", name="psv")
            for cc in range(CCH):
                nc.tensor.matmul(
                    out=ps,
                    lhsT=xT[cc][:, ti * 128:(ti + 1) * 128],
                    rhs=wv_sb[jvt][cc],
                    start=(cc == 0), stop=(cc == CCH - 1))
            nc.vector.tensor_tensor(
                out=vcol[:, jvt * 8:(jvt + 1) * 8, 0:D],
                in0=ps.rearrange("p (h d) -> p h d", h=8),
                in1=bvb[:, jvt * 512:(jvt + 1) * 512].rearrange(
                    "p (h d) -> p h d", h=8),
                op=ADD)

        def outproj(ti, half):
            sl = slice(half * 512, (half + 1) * 512)
            ps = P1.tile([128, 512], FP32, tag="p1", name="pso")
            for hc in range(CCH):
                nc.tensor.matmul(
                    out=ps,
                    lhsT=oT[hc][:, ti * 128:(ti + 1) * 128],
                    rhs=wo_sb[half][:, hc, :],
                    start=(hc == 0), stop=(hc == CCH - 1))
            ot = outs.tile([128, 512], FP32, tag="ot", name="ot")
            nc.vector.tensor_tensor(out=ot, in0=ps, in1=bob[:, sl], op=ADD)
            nc.sync.dma_start(
                out=out_d[ti * 128:(ti + 1) * 128, sl], in_=ot)

        # ---- phase A ----
        for jt in (0, 8, 1, 9, 2, 10):
            qk_group(jt, 0)
        for ti in range(TCH):
            vproj(ti, 0)
        for ti in range(TCH):
            vproj(ti, 1)

        # ---- phase B: attention, it-outer, with PE filler interleave ----
        def make_attnv(hp, jc, njc, pT, s0, po2):
            def go():
                if jc == 0:
                    # allocate lazily at emission time so the ring-slot reuse
                    # dependency sees the previous pair's (lagged) norm reads
                    po2[0] = P3.tile([65, 512], FP32, tag="po0", name="po0")
                    po2[1] = P3.tile([65, 512], FP32, tag="po1", name="po1")
                for hx, h in enumerate((2 * hp, 2 * hp + 1)):
                    hsl = slice(h * (D + 1), h * (D + 1) + D + 1)
                    nc.tensor.matmul(
                        out=po2[hx][0:65, s0:512],
                        lhsT=vp[jc][:, hsl],
                        rhs=pT[:, hx, s0:512],
                        start=(jc == 0), stop=(jc == njc - 1),
                        skip_group_check=True)
            return go

        def make_norm(hp, it, po2):
            def go():
                for hx in range(2):
                    l_sb = lpool.tile([1, 512], FP32, tag="l", name="l")
                    # custom-DVE ops misread partition-offset inputs; extract
                    # the l row with a plain copy first
                    nc.vector.tensor_copy(out=l_sb, in_=po2[hx][64:65, :])
                    rb1 = lpool.tile([1, 512], FP32, tag="rb1", name="rb1")
                    nc.vector.reciprocal_approx_fast(out=rb1, in_=l_sb)
                    rb = rbpool.tile([64, 512], FP32, tag="rb", name="rb")
                    nc.gpsimd.partition_broadcast(out_ap=rb, in_ap=rb1)
                    nc.vector.tensor_tensor(
                        out=oT[hp][hx * 64:(hx + 1) * 64,
                                   it * 512:(it + 1) * 512],
                        in0=po2[hx][0:64, :], in1=rb, op=MULT)
            return go

        # filler schedules: {unit_index: [closures]}
        f_b1 = {}
        b1q = [(3, 0), (11, 0), (4, 0), (12, 0), (5, 0), (13, 0), (6, 0),
               (14, 0), (7, 0), (15, 0), (0, 1), (8, 1), (1, 1), (9, 1)]
        for u, g in enumerate(b1q):
            f_b1.setdefault(2 * u, []).append(g)
        f_b2 = {}
        b2q = [(2, 1), (10, 1), (3, 1), (11, 1), (4, 1), (12, 1), (5, 1),
               (13, 1), (6, 1), (14, 1), (7, 1), (15, 1)]
        for u, g in enumerate(b2q):
            f_b2.setdefault(u, []).append(g)
        b2o = [(0, 0), (0, 1), (1, 0), (1, 1), (2, 0), (2, 1), (3, 0), (3, 1)]
        for i, o in enumerate(b2o):
            f_b2.setdefault(24 + 3 * i, []).append(("OUT", o))

        LAG = 2
        pending = deque()

        def flush(n_keep):
            while len(pending) > n_keep:
                pending.popleft()()

        for it in (0, 1):
            fill = f_b1 if it == 0 else f_b2
            njc = 4 * (it + 1)
            u = 0
            for hp in range(8):
                po2 = [None, None]
                for jc in range(njc):
                    s0 = max(0, jc * 128 - it * 512)
                    ps = P2.tile([128, 2, 512], FP32, tag="ps", name="ps")
                    pT = ppool.tile([128, 2, 512], BF16, tag="pT", name="pT")
                    for hx in range(2):
                        prow = slice(hx * 64, hx * 64 + 64)
                        nc.tensor.matmul(
                            out=ps[:, hx, s0:512],
                            lhsT=qkT[8 + hp][prow, jc * 128:(jc + 1) * 128],
                            rhs=qkT[hp][prow, it * 512 + s0:(it + 1) * 512],
                            start=True, stop=True)
                    nc.scalar.activation(
                        out=pT[:, :, s0:512], in_=ps[:, :, s0:512],
                        func=mybir.ActivationFunctionType.Exp, scale=0.125)
                    if jc >= it * 4:  # diagonal block: post-exp causal zeroing
                        nc.gpsimd.affine_select(
                            out=pT[:, :, s0:s0 + 128],
                            in_=pT[:, :, s0:s0 + 128],
                            compare_op=mybir.AluOpType.is_ge, fill=0.0,
                            base=0, channel_multiplier=-1,
                            pattern=[[0, 2], [1, 128]])
                    for f in fill.get(u, ()):
                        if isinstance(f, tuple) and f and f[0] == "OUT":
                            outproj(*f[1])
                        else:
                            qk_group(*f)
                    pending.append(make_attnv(hp, jc, njc, pT, s0, po2))
                    flush(LAG)
                    u += 1
                pending.append(make_norm(hp, it, po2))
            flush(0)

        # ---- phase C: out projection, token tiles 4..7 ----
        for ti in range(4, TCH):
            for half in range(2):
                outproj(ti, half)

    nc.compile()
    nc.finalize()
    return nc


_CACHE = {}


def _bf16(a):
    return np.ascontiguousarray(np.asarray(a, np.float32)).astype(
        ml_dtypes.bfloat16)


def _fp8(a):
    return np.ascontiguousarray(
        np.clip(np.asarray(a, np.float32), -240.0, 240.0)).astype(
        ml_dtypes.float8_e4m3)


def make_in_maps(x, W_qkv, b_qkv, W_out, b_out):
    W_qkv = np.asarray(W_qkv, np.float32)
    W_out = np.asarray(W_out, np.float32)
    b_qkv = np.asarray(b_qkv, np.float32).reshape(-1)
    b_out = np.asarray(b_out, np.float32).reshape(1, -1)
    x = np.asarray(x, np.float32)

    # fp8 q/k weights, 32x scaled: [16 jt][128 part, 4 pair, 2, 128 col]
    # element [part, p, e, col] = 32*Wqkv[128*(2p+e)+part, 128*jt+col]
    wq8 = np.stack([
        _fp8((WS * W_qkv[:, jt * 128:(jt + 1) * 128])
             .reshape(NPAIR, 2, 128, 128).transpose(2, 0, 1, 3))
        for jt in range(16)])
    wv = np.stack([
        _bf16(W_qkv[:, 2 * C + j * 512:2 * C + (j + 1) * 512]
              .reshape(CCH, 128, 512).transpose(1, 0, 2))
        for j in range(2)])
    wo = np.stack([
        _bf16(W_out[:, h * 512:(h + 1) * 512]
              .reshape(CCH, 128, 512).transpose(1, 0, 2))
        for h in range(2)])
    bqT = np.ascontiguousarray(b_qkv[:2 * C].reshape(16, 128).T)
    bqv = np.ascontiguousarray(b_qkv[2 * C:].reshape(1, C))

    shared = {
        "wq8": wq8, "wv": wv, "wo": wo,
        "bqT": bqT, "bqv": bqv, "bo": np.ascontiguousarray(b_out),
    }
    maps = []
    for i in range(N_CORES):
        xT = x[i].T  # [C, T]
        # x8 pairs: [4, 128, 2, T]: [p, part, e, t] = x[t, 128*(2p+e)+part]
        x8 = _fp8(xT.reshape(NPAIR, 2, 128, T).transpose(0, 2, 1, 3))
        maps.append(dict(shared, x8_b=x8, xT_b=_bf16(xT)))
    return maps


def kernel(x, W_qkv, b_qkv, W_out, b_out):
    if "nc" not in _CACHE:
        _CACHE["nc"] = build_nc()
    nc = _CACHE["nc"]
    in_maps = make_in_maps(x, W_qkv, b_qkv, W_out, b_out)
    res = bass_utils.run_bass_kernel_spmd(nc, in_maps, core_ids=list(range(N_CORES)))
    return np.stack([r["out_b"] for r in res.results]).astype(np.float32)


# revision 14
# speedup vs baseline: 1.0812x; 1.0264x over previous
# Causal self-attention (B=8, T=1024, C=1024, H=16, D=64) on 8 trn2 NeuronCores.
# Sharding: data-parallel over batch — core i computes batch element i entirely
# (weights replicated, no collectives).
#
# v2: Q/K projections run as fp8e4 DoubleRow matmuls (two 128-deep contraction
# chunks per pass). Weights are pre-scaled by 32 on the host so their +-1/32
# range maps into fp8 normal range; the DVE consumer rescales by 1/32 while
# adding the bias (tensor_scalar mult+add). Softmax renormalization absorbs the
# q/k quantization error (measured ~0.8% end-to-end). The v / attn / out-proj
# paths stay bf16 — fp8 there costs ~4% end-to-end which exceeds tolerance.
#
# Per-core pipeline (fp32 PSUM accumulation everywhere):
#   A: qk groups for head-pairs 0,1 (fp8 DR), v[t,j] per-head with a leading
#      denominator column (width 65, ones-first so the softmax-l row lands at
#      PSUM partition 0 where the custom-DVE reciprocal can read it directly).
#   B: for it (query 512-tile) 0,1; head pair hp; key chunk jc:
#      scores sT[j,i] (K=64 pair, row bases 0/64 -> auto row-tiled, concurrent),
#      exp on ACT (scale=1/8) -> pT bf16, gpsimd affine_select causal mask on
#      the diagonal 128-block, attn@v (M=65; row 0 = l = sum_j p) with a 2-unit
#      software-pipeline lag. Normalize: DVE reciprocal of l row (direct from
#      PSUM), gpsimd partition_broadcast, DVE multiply -> oT bf16.
#      Remaining qk groups + out-proj token tiles 0..3 interleave as PE filler.
#   C: out-proj token tiles 4..7 -> DMA to DRAM per (ti, half).

from collections import deque
from contextlib import ExitStack

import numpy as np
import ml_dtypes

import concourse.bass as bass
import concourse.bacc as bacc
import concourse.mybir as mybir
import concourse.tile as tile
from concourse import bass_utils

FP32 = mybir.dt.float32
BF16 = mybir.dt.bfloat16
FP8 = mybir.dt.float8e4

B, T, C = 8, 1024, 1024
H, D = 16, 64
N_CORES = 8
CCH = C // 128   # 8 contraction chunks of 128
TCH = T // 128   # 8 token chunks of 128
NPAIR = 4        # 4 DoubleRow pairs of contraction chunks
WS = 32.0        # host-side fp8 weight scale (power of two)
DR = mybir.MatmulPerfMode.DoubleRow


def build_nc():
    nc = bacc.Bacc("TRN2", debug=False, num_devices=N_CORES)

    x8_d = nc.dram_tensor("x8_b", [NPAIR, 128, 2, T], FP8, kind="ExternalInput").ap()
    xT_d = nc.dram_tensor("xT_b", [C, T], BF16, kind="ExternalInput").ap()
    wq8_d = nc.dram_tensor("wq8", [16, 128, NPAIR, 2, 128], FP8,
                           kind="ExternalInput").ap()
    wv_d = nc.dram_tensor("wv", [2, 128, CCH, 512], BF16, kind="ExternalInput").ap()
    wo_d = nc.dram_tensor("wo", [2, 128, CCH, 512], BF16, kind="ExternalInput").ap()
    bqT_d = nc.dram_tensor("bqT", [128, 16], FP32, kind="ExternalInput").ap()
    bqv_d = nc.dram_tensor("bqv", [1, C], FP32, kind="ExternalInput").ap()
    bo_d = nc.dram_tensor("bo", [1, C], FP32, kind="ExternalInput").ap()
    out_d = nc.dram_tensor("out_b", [T, C], FP32, kind="ExternalOutput").ap()

    ADD = mybir.AluOpType.add
    MULT = mybir.AluOpType.mult

    with tile.TileContext(nc) as tc, ExitStack() as ctx:
        consts = ctx.enter_context(tc.tile_pool(name="consts", bufs=1))
        wpool = ctx.enter_context(tc.tile_pool(name="weights", bufs=1))
        apool = ctx.enter_context(tc.tile_pool(name="acts", bufs=1))
        ppool = ctx.enter_context(tc.tile_pool(name="ppool", bufs=4))
        lpool = ctx.enter_context(tc.tile_pool(name="lpool", bufs=2))
        rbpool = ctx.enter_context(tc.tile_pool(name="rbpool", bufs=2))
        outs = ctx.enter_context(tc.tile_pool(name="outs", bufs=3))
        # PSUM (8 banks): P1 2x[128,512]=2, P2 2x[128,2,512]=4, P3 2x[65,512]=2
        P1 = ctx.enter_context(tc.tile_pool(name="P1", bufs=2, space="PSUM"))
        P2 = ctx.enter_context(tc.tile_pool(name="P2", bufs=2, space="PSUM"))
        P3 = ctx.enter_context(tc.tile_pool(name="P3", bufs=1, space="PSUM"))

        # ---- tiny const DMAs first ----
        bqv_sb = consts.tile([1, C], FP32, tag="bqv")
        nc.gpsimd.dma_start(out=bqv_sb, in_=bqv_d)
        bo_sb = consts.tile([1, C], FP32, tag="bo")
        nc.gpsimd.dma_start(out=bo_sb, in_=bo_d)
        bqT = consts.tile([128, 16], FP32, tag="bqT")
        nc.gpsimd.dma_start(out=bqT, in_=bqT_d)

        # ---- persistent activations ----
        x8 = [apool.tile([128, 2, T], FP8, tag=f"x8{p}", name=f"x8{p}")
              for p in range(NPAIR)]
        xT = [apool.tile([128, T], BF16, tag=f"xT{cc}", name=f"xT{cc}")
              for cc in range(CCH)]
        qkT = [apool.tile([128, T], BF16, tag=f"qkT{jt}", name=f"qkT{jt}")
               for jt in range(16)]
        vp = [apool.tile([128, H * (D + 1)], BF16, tag=f"vp{t_}", name=f"vp{t_}")
              for t_ in range(TCH)]
        oT = [apool.tile([128, T], BF16, tag=f"oT{hc}", name=f"oT{hc}")
              for hc in range(CCH)]

        # ---- DMAs (3 issue queues: sync / scalar / gpsimd).  x8 + wq8[0,8]
        # first (unblocks phase A), then xT/wv (v path), then the rest in
        # consumption order; wo (needed ~40us in) last. ----
        wq8_sb = [wpool.tile([128, NPAIR, 2, 128], FP8, tag=f"wq{jt}",
                             name=f"wq{jt}") for jt in range(16)]
        # wv split per contraction chunk so vproj's cc-loop can start early
        wv_sb = [[wpool.tile([128, 512], BF16, tag=f"wv{j}_{cc}",
                             name=f"wv{j}_{cc}") for cc in range(CCH)]
                 for j in range(2)]
        wo_sb = [wpool.tile([128, CCH, 512], BF16, tag=f"wo{h}", name=f"wo{h}")
                 for h in range(2)]

        # sync carries the bulk (dedicated queue engine); scalar stays light
        # so the ACT table-load + first exp aren't stuck behind DMA issues;
        # gpsimd stays light so affine_selects/broadcasts aren't delayed
        # (each DMA issue costs ~630ns of queue-engine time).
        nc.sync.dma_start(out=x8[0], in_=x8_d[0])
        nc.scalar.dma_start(out=x8[1], in_=x8_d[1])
        nc.sync.dma_start(out=x8[2], in_=x8_d[2])
        nc.scalar.dma_start(out=x8[3], in_=x8_d[3])
        nc.sync.dma_start(out=wq8_sb[0], in_=wq8_d[0])
        nc.scalar.dma_start(out=wq8_sb[8], in_=wq8_d[8])
        nc.sync.dma_start(out=wq8_sb[1], in_=wq8_d[1])
        nc.sync.dma_start(out=wq8_sb[9], in_=wq8_d[9])
        for cc in range(0, CCH, 2):
            nc.scalar.dma_start(out=xT[cc], in_=xT_d[cc * 128:(cc + 1) * 128, :])
            nc.sync.dma_start(out=xT[cc + 1],
                              in_=xT_d[(cc + 1) * 128:(cc + 2) * 128, :])
        nc.scalar.dma_start(out=wq8_sb[2], in_=wq8_d[2])
        nc.scalar.dma_start(out=wq8_sb[10], in_=wq8_d[10])
        for cc in range(CCH):
            nc.sync.dma_start(out=wv_sb[0][cc], in_=wv_d[0, :, cc, :])
        for cc in range(CCH):
            nc.sync.dma_start(out=wv_sb[1][cc], in_=wv_d[1, :, cc, :])
        for jt in (3, 11, 5, 13):
            nc.sync.dma_start(out=wq8_sb[jt], in_=wq8_d[jt])
        for jt in (4, 12, 6, 14, 7, 15):
            nc.gpsimd.dma_start(out=wq8_sb[jt], in_=wq8_d[jt])
        nc.gpsimd.dma_start(out=wo_sb[0], in_=wo_d[0])
        nc.gpsimd.dma_start(out=wo_sb[1], in_=wo_d[1])

        # ---- bias broadcasts (gpsimd; no PSUM, no PE) ----
        bvb = consts.tile([128, C], FP32, tag="bvb")
        nc.gpsimd.partition_broadcast(out_ap=bvb, in_ap=bqv_sb)
        bob = consts.tile([128, C], FP32, tag="bob")
        nc.gpsimd.partition_broadcast(out_ap=bob, in_ap=bo_sb)

        # denominator (ones) columns of vp, once.  Ones sit LAST within each
        # head's 65-wide block: PSUM partition access must be 32-aligned, so
        # the norm-mult must read po[0:64] (offset 0) and l lives at row 64.
        for ti in range(TCH):
            vcol = vp[ti].rearrange("p (h d) -> p h d", h=H)
            nc.vector.memset(vcol[:, :, D:D + 1], 1.0)

        # ---- building blocks ----
        def qk_group(jt, half):
            sl = slice(half * 512, (half + 1) * 512)
            ps = P1.tile([128, 512], FP32, tag="p1", name="psqk")
            for p in range(NPAIR):
                nc.tensor.matmul(
                    out=ps,
                    lhsT=wq8_sb[jt][:, p],
                    rhs=x8[p][:, :, sl],
                    start=(p == 0), stop=(p == NPAIR - 1),
                    perf_mode=DR)
            nc.vector.tensor_scalar(
                out=qkT[jt][:, sl], in0=ps,
                scalar1=1.0 / WS, scalar2=bqT[:, jt:jt + 1],
                op0=MULT, op1=ADD)

        def vproj(ti, jvt):
            vcol = vp[ti].rearrange("p (h d) -> p h d", h=H)
            ps = P1.tile([128, 512], FP32, tag="p1", name="psv")
            for cc in range(CCH):
                nc.tensor.matmul(
                    out=ps,
                    lhsT=xT[cc][:, ti * 128:(ti + 1) * 128],
                    rhs=wv_sb[jvt][cc],
                    start=(cc == 0), stop=(cc == CCH - 1))
            nc.vector.tensor_tensor(
                out=vcol[:, jvt * 8:(jvt + 1) * 8, 0:D],
                in0=ps.rearrange("p (h d) -> p h d", h=8),
                in1=bvb[:, jvt * 512:(jvt + 1) * 512].rearrange(
                    "p (h d) -> p h d", h=8),
                op=ADD)

        def outproj(ti, half):
            sl = slice(half * 512, (half + 1) * 512)
            ps = P1.tile([128, 512], FP32, tag="p1", name="pso")
            for hc in range(CCH):
                nc.tensor.matmul(
                    out=ps,
                    lhsT=oT[hc][:, ti * 128:(ti + 1) * 128],
                    rhs=wo_sb[half][:, hc, :],
                    start=(hc == 0), stop=(hc == CCH - 1))
            ot = outs.tile([128, 512], FP32, tag="ot", name="ot")
            nc.vector.tensor_tensor(out=ot, in0=ps, in1=bob[:, sl], op=ADD)
            nc.sync.dma_start(
                out=out_d[ti * 128:(ti + 1) * 128, sl], in_=ot)

        # ---- phase A (minimal: just what scores hp0 needs; everything else
        # streams in as phase-B filler) ----
        qk_group(0, 0)
        qk_group(8, 0)

        # ---- phase B: attention, it-outer, with PE filler interleave ----
        def make_attnv(hp, jc, njc, pT, s0, po2):
            def go():
                if jc == 0:
                    # allocate lazily at emission time so the ring-slot reuse
                    # dependency sees the previous pair's (lagged) norm reads
                    po2[0] = P3.tile([65, 512], FP32, tag="po0", name="po0")
                    po2[1] = P3.tile([65, 512], FP32, tag="po1", name="po1")
                for hx, h in enumerate((2 * hp, 2 * hp + 1)):
                    hsl = slice(h * (D + 1), h * (D + 1) + D + 1)
                    nc.tensor.matmul(
                        out=po2[hx][0:65, s0:512],
                        lhsT=vp[jc][:, hsl],
                        rhs=pT[:, hx, s0:512],
                        start=(jc == 0), stop=(jc == njc - 1),
                        skip_group_check=True)
            return go

        def make_norm(hp, it, po2):
            def go():
                # both hx chains interleaved so the gpsimd broadcast of hx0
                # overlaps the DVE copy/recip of hx1 (critical at flush tails)
                l_sb, rb1, rb = [None, None], [None, None], [None, None]
                for hx in range(2):
                    l_sb[hx] = lpool.tile([1, 512], FP32, tag="l", name="l")
                    # custom-DVE ops misread partition-offset inputs; extract
                    # the l row with a plain copy first
                    nc.vector.tensor_copy(out=l_sb[hx], in_=po2[hx][64:65, :])
                for hx in range(2):
                    rb1[hx] = lpool.tile([1, 512], FP32, tag="rb1", name="rb1")
                    nc.vector.reciprocal_approx_fast(out=rb1[hx], in_=l_sb[hx])
                    rb[hx] = rbpool.tile([64, 512], FP32, tag="rb", name="rb")
                    nc.gpsimd.partition_broadcast(out_ap=rb[hx], in_ap=rb1[hx])
                for hx in range(2):
                    nc.vector.tensor_tensor(
                        out=oT[hp][hx * 64:(hx + 1) * 64,
                                   it * 512:(it + 1) * 512],
                        in0=po2[hx][0:64, :], in1=rb[hx], op=MULT)
            return go

        # filler schedules: {unit_index: [(kind, args)]}
        # it0 (32 units): remaining qk halves in scores-consumption order +
        # all 16 vprojs (vp[jc] jvt0 needed by attnv hp<4; jvt1 by hp>=4)
        # NOTE: a filler must be EMITTED (unit index) before any lagged attnv
        # that reads its output pops from the pending queue — the tile
        # tracker orders by emission, so a late write is an untracked race.
        # attnv(hp0, jc) pops at unit jc+2 -> vproj(jc, 0) must sit at unit
        # <= jc+1; scores(hp, jc0) needs its qk halves strictly earlier.
        f_b1 = {}
        b1 = [(0, ("V", (0, 0))), (0, ("QK", (1, 0))),
              (1, ("V", (1, 0))), (1, ("QK", (9, 0))),
              (2, ("V", (2, 0))), (3, ("V", (3, 0))),
              (4, ("QK", (2, 0))), (5, ("QK", (10, 0))),
              (6, ("V", (0, 1))), (7, ("V", (1, 1))),
              (8, ("QK", (3, 0))), (9, ("QK", (11, 0))),
              (10, ("V", (2, 1))), (11, ("V", (3, 1))),
              (12, ("QK", (4, 0))), (13, ("QK", (12, 0))),
              (14, ("QK", (5, 0))), (15, ("QK", (13, 0))),
              (16, ("QK", (6, 0))), (17, ("QK", (14, 0))),
              (18, ("QK", (7, 0))), (19, ("QK", (15, 0))),
              (20, ("QK", (0, 1))), (21, ("QK", (8, 1))),
              (22, ("QK", (1, 1))), (23, ("QK", (9, 1)))]
        for u, g in b1:
            f_b1.setdefault(u, []).append(g)
        # it1 (64 units): vp[4..7], remaining qk h1 halves, out-proj t 0:512
        f_b2 = {}
        b2 = [("V", (4, 0)), ("V", (5, 0)), ("V", (6, 0)), ("V", (7, 0)),
              ("QK", (2, 1)), ("QK", (10, 1)), ("V", (4, 1)), ("V", (5, 1)),
              ("QK", (3, 1)), ("QK", (11, 1)), ("V", (6, 1)), ("V", (7, 1)),
              ("QK", (4, 1)), ("QK", (12, 1)), ("QK", (5, 1)), ("QK", (13, 1)),
              ("QK", (6, 1)), ("QK", (14, 1)), ("QK", (7, 1)), ("QK", (15, 1))]
        for u, g in enumerate(b2):
            f_b2.setdefault(u, []).append(g)
        b2o = [(0, 0), (0, 1), (1, 0), (1, 1), (2, 0), (2, 1), (3, 0), (3, 1)]
        for i, o in enumerate(b2o):
            f_b2.setdefault(24 + 3 * i, []).append(("OUT", o))

        LAG = 2
        pending = deque()

        def flush(n_keep):
            while len(pending) > n_keep:
                pending.popleft()()

        for it in (0, 1):
            fill = f_b1 if it == 0 else f_b2
            njc = 4 * (it + 1)
            u = 0
            for hp in range(8):
                po2 = [None, None]
                for jc in range(njc):
                    s0 = max(0, jc * 128 - it * 512)
                    ps = P2.tile([128, 2, 512], FP32, tag="ps", name="ps")
                    pT = ppool.tile([128, 2, 512], BF16, tag="pT", name="pT")
                    for hx in range(2):
                        prow = slice(hx * 64, hx * 64 + 64)
                        nc.tensor.matmul(
                            out=ps[:, hx, s0:512],
                            lhsT=qkT[8 + hp][prow, jc * 128:(jc + 1) * 128],
                            rhs=qkT[hp][prow, it * 512 + s0:(it + 1) * 512],
                            start=True, stop=True)
                    nc.scalar.activation(
                        out=pT[:, :, s0:512], in_=ps[:, :, s0:512],
                        func=mybir.ActivationFunctionType.Exp, scale=0.125)
                    if jc >= it * 4:  # diagonal block: post-exp causal zeroing
                        nc.gpsimd.affine_select(
                            out=pT[:, :, s0:s0 + 128],
                            in_=pT[:, :, s0:s0 + 128],
                            compare_op=mybir.AluOpType.is_ge, fill=0.0,
                            base=0, channel_multiplier=-1,
                            pattern=[[0, 2], [1, 128]])
                    for kind, args in fill.get(u, ()):
                        if kind == "OUT":
                            outproj(*args)
                        elif kind == "V":
                            vproj(*args)
                        else:
                            qk_group(*args)
                    pending.append(make_attnv(hp, jc, njc, pT, s0, po2))
                    flush(LAG)
                    u += 1
                pending.append(make_norm(hp, it, po2))
            flush(0)

        # ---- phase C: out projection, token tiles 4..7 ----
        for ti in range(4, TCH):
            for half in range(2):
                outproj(ti, half)

    nc.compile()
    nc.finalize()
    return nc


_CACHE = {}


def _bf16(a):
    return np.ascontiguousarray(np.asarray(a, np.float32)).astype(
        ml_dtypes.bfloat16)


def _fp8(a):
    return np.ascontiguousarray(
        np.clip(np.asarray(a, np.float32), -240.0, 240.0)).astype(
        ml_dtypes.float8_e4m3)


def make_in_maps(x, W_qkv, b_qkv, W_out, b_out):
    W_qkv = np.asarray(W_qkv, np.float32)
    W_out = np.asarray(W_out, np.float32)
    b_qkv = np.asarray(b_qkv, np.float32).reshape(-1)
    b_out = np.asarray(b_out, np.float32).reshape(1, -1)
    x = np.asarray(x, np.float32)

    # fp8 q/k weights, 32x scaled: [16 jt][128 part, 4 pair, 2, 128 col]
    # element [part, p, e, col] = 32*Wqkv[128*(2p+e)+part, 128*jt+col]
    wq8 = np.stack([
        _fp8((WS * W_qkv[:, jt * 128:(jt + 1) * 128])
             .reshape(NPAIR, 2, 128, 128).transpose(2, 0, 1, 3))
        for jt in range(16)])
    wv = np.stack([
        _bf16(W_qkv[:, 2 * C + j * 512:2 * C + (j + 1) * 512]
              .reshape(CCH, 128, 512).transpose(1, 0, 2))
        for j in range(2)])
    wo = np.stack([
        _bf16(W_out[:, h * 512:(h + 1) * 512]
              .reshape(CCH, 128, 512).transpose(1, 0, 2))
        for h in range(2)])
    bqT = np.ascontiguousarray(b_qkv[:2 * C].reshape(16, 128).T)
    bqv = np.ascontiguousarray(b_qkv[2 * C:].reshape(1, C))

    shared = {
        "wq8": wq8, "wv": wv, "wo": wo,
        "bqT": bqT, "bqv": bqv, "bo": np.ascontiguousarray(b_out),
    }
    maps = []
    for i in range(N_CORES):
        xT = x[i].T  # [C, T]
        # x8 pairs: [4, 128, 2, T]: [p, part, e, t] = x[t, 128*(2p+e)+part]
        x8 = _fp8(xT.reshape(NPAIR, 2, 128, T).transpose(0, 2, 1, 3))
        maps.append(dict(shared, x8_b=x8, xT_b=_bf16(xT)))
    return maps


def kernel(x, W_qkv, b_qkv, W_out, b_out):
    if "nc" not in _CACHE:
        _CACHE["nc"] = build_nc()
    nc = _CACHE["nc"]
    in_maps = make_in_maps(x, W_qkv, b_qkv, W_out, b_out)
    res = bass_utils.run_bass_kernel_spmd(nc, in_maps, core_ids=list(range(N_CORES)))
    return np.stack([r["out_b"] for r in res.results]).astype(np.float32)


# revision 24
# speedup vs baseline: 1.1469x; 1.0608x over previous
# Causal self-attention (B=8, T=1024, C=1024, H=16, D=64) on 8 trn2 NeuronCores.
# Sharding: data-parallel over batch — core i computes batch element i entirely
# (weights replicated, no collectives).
#
# v2: Q/K projections run as fp8e4 DoubleRow matmuls (two 128-deep contraction
# chunks per pass). Weights are pre-scaled by 32 on the host so their +-1/32
# range maps into fp8 normal range; the DVE consumer rescales by 1/32 while
# adding the bias (tensor_scalar mult+add). Softmax renormalization absorbs the
# q/k quantization error (measured ~0.8% end-to-end). The v / attn / out-proj
# paths stay bf16 — fp8 there costs ~4% end-to-end which exceeds tolerance.
#
# Per-core pipeline (fp32 PSUM accumulation everywhere):
#   A: qk groups for head-pairs 0,1 (fp8 DR), v[t,j] per-head with a leading
#      denominator column (width 65, ones-first so the softmax-l row lands at
#      PSUM partition 0 where the custom-DVE reciprocal can read it directly).
#   B: for it (query 512-tile) 0,1; head pair hp; key chunk jc:
#      scores sT[j,i] (K=64 pair, row bases 0/64 -> auto row-tiled, concurrent),
#      exp on ACT (scale=1/8) -> pT bf16, gpsimd affine_select causal mask on
#      the diagonal 128-block, attn@v (M=65; row 0 = l = sum_j p) with a 2-unit
#      software-pipeline lag. Normalize: DVE reciprocal of l row (direct from
#      PSUM), gpsimd partition_broadcast, DVE multiply -> oT bf16.
#      Remaining qk groups + out-proj token tiles 0..3 interleave as PE filler.
#   C: out-proj token tiles 4..7 -> DMA to DRAM per (ti, half).

from collections import deque
from contextlib import ExitStack

import numpy as np
import ml_dtypes

import concourse.bass as bass
import concourse.bacc as bacc
import concourse.mybir as mybir
import concourse.tile as tile
from concourse import bass_utils

FP32 = mybir.dt.float32
BF16 = mybir.dt.bfloat16
FP8 = mybir.dt.float8e4

B, T, C = 8, 1024, 1024
H, D = 16, 64
N_CORES = 8
CCH = C // 128   # 8 contraction chunks of 128
TCH = T // 128   # 8 token chunks of 128
NPAIR = 4        # 4 DoubleRow pairs of contraction chunks
WS = 32.0        # host-side fp8 weight scale (power of two)
DR = mybir.MatmulPerfMode.DoubleRow


def build_nc():
    nc = bacc.Bacc("TRN2", debug=False, num_devices=N_CORES)

    x8_d = nc.dram_tensor("x8_b", [NPAIR, 128, 2, T], FP8, kind="ExternalInput").ap()
    xT_d = nc.dram_tensor("xT_b", [C, T], BF16, kind="ExternalInput").ap()
    wq8_d = nc.dram_tensor("wq8", [16, 128, NPAIR, 2, 128], FP8,
                           kind="ExternalInput").ap()
    wv_d = nc.dram_tensor("wv", [2, CCH, 128, 512], BF16, kind="ExternalInput").ap()
    wo_d = nc.dram_tensor("wo", [2, 128, CCH, 512], BF16, kind="ExternalInput").ap()
    bqT_d = nc.dram_tensor("bqT", [128, 16], FP32, kind="ExternalInput").ap()
    bqv_d = nc.dram_tensor("bqv", [1, C], FP32, kind="ExternalInput").ap()
    bo_d = nc.dram_tensor("bo", [1, C], FP32, kind="ExternalInput").ap()
    out_d = nc.dram_tensor("out_b", [T, C], FP32, kind="ExternalOutput").ap()

    ADD = mybir.AluOpType.add
    MULT = mybir.AluOpType.mult

    with tile.TileContext(nc) as tc, ExitStack() as ctx:
        consts = ctx.enter_context(tc.tile_pool(name="consts", bufs=1))
        wpool = ctx.enter_context(tc.tile_pool(name="weights", bufs=1))
        apool = ctx.enter_context(tc.tile_pool(name="acts", bufs=1))
        ppool = ctx.enter_context(tc.tile_pool(name="ppool", bufs=6))
        lpool = ctx.enter_context(tc.tile_pool(name="lpool", bufs=2))
        rbpool = ctx.enter_context(tc.tile_pool(name="rbpool", bufs=2))
        outs = ctx.enter_context(tc.tile_pool(name="outs", bufs=3))
        # PSUM (8 banks): P1 2x[128,512]=2, P2 2x[128,2,512]=4, P3 2x[65,512]=2
        P1 = ctx.enter_context(tc.tile_pool(name="P1", bufs=2, space="PSUM"))
        P2 = ctx.enter_context(tc.tile_pool(name="P2", bufs=2, space="PSUM"))
        P3 = ctx.enter_context(tc.tile_pool(name="P3", bufs=1, space="PSUM"))

        # ---- tiny const DMAs first; broadcasts immediately after.
        # A consumer waits for ALL earlier DMAs on the same issue queue, so
        # gpsimd carries ONLY these three tiny DMAs — anything heavier here
        # would stall the broadcasts (and with them the causal masks). ----
        bqv_sb = consts.tile([1, C], FP32, tag="bqv")
        nc.gpsimd.dma_start(out=bqv_sb, in_=bqv_d)
        bo_sb = consts.tile([1, C], FP32, tag="bo")
        nc.gpsimd.dma_start(out=bo_sb, in_=bo_d)
        bqT = consts.tile([128, 16], FP32, tag="bqT")
        nc.gpsimd.dma_start(out=bqT, in_=bqT_d)
        bvb = consts.tile([128, C], FP32, tag="bvb")
        nc.gpsimd.partition_broadcast(out_ap=bvb, in_ap=bqv_sb)
        bob = consts.tile([128, C], FP32, tag="bob")
        nc.gpsimd.partition_broadcast(out_ap=bob, in_ap=bo_sb)

        # ---- persistent activations ----
        x8 = [apool.tile([128, 2, T], FP8, tag=f"x8{p}", name=f"x8{p}")
              for p in range(NPAIR)]
        xT = [apool.tile([128, T], BF16, tag=f"xT{cc}", name=f"xT{cc}")
              for cc in range(CCH)]
        qkT = [apool.tile([128, T], BF16, tag=f"qkT{jt}", name=f"qkT{jt}")
               for jt in range(16)]
        vp = [apool.tile([128, H * (D + 1)], BF16, tag=f"vp{t_}", name=f"vp{t_}")
              for t_ in range(TCH)]
        oT = [apool.tile([128, T], BF16, tag=f"oT{hc}", name=f"oT{hc}")
              for hc in range(CCH)]

        # ---- DMAs (3 issue queues: sync / scalar / gpsimd).  x8 + wq8[0,8]
        # first (unblocks phase A), then xT/wv (v path), then the rest in
        # consumption order; wo (needed ~40us in) last. ----
        wq8_sb = [wpool.tile([128, NPAIR, 2, 128], FP8, tag=f"wq{jt}",
                             name=f"wq{jt}") for jt in range(16)]
        # wv split per contraction chunk so vproj's cc-loop can start early
        wv_sb = [[wpool.tile([128, 512], BF16, tag=f"wv{j}_{cc}",
                             name=f"wv{j}_{cc}") for cc in range(CCH)]
                 for j in range(2)]
        wo_sb = [wpool.tile([128, CCH, 512], BF16, tag=f"wo{h}", name=f"wo{h}")
                 for h in range(2)]

        # sync + scalar carry the bulk, each in strict consumption order
        # (per-queue completion is ordered, so a late-needed DMA placed early
        # delays every later consumer on that queue).  scalar's total stays
        # modest so its DMA issues clear before the first exp.
        nc.sync.dma_start(out=x8[0], in_=x8_d[0])
        nc.scalar.dma_start(out=x8[1], in_=x8_d[1])
        nc.sync.dma_start(out=x8[2], in_=x8_d[2])
        nc.scalar.dma_start(out=x8[3], in_=x8_d[3])
        nc.sync.dma_start(out=wq8_sb[0], in_=wq8_d[0])
        nc.scalar.dma_start(out=wq8_sb[8], in_=wq8_d[8])
        nc.sync.dma_start(out=wq8_sb[1], in_=wq8_d[1])
        nc.scalar.dma_start(out=wq8_sb[9], in_=wq8_d[9])
        # v path: xT + wv0 split across both queues, interleaved per chunk
        for cc in range(0, CCH, 2):
            nc.sync.dma_start(out=xT[cc], in_=xT_d[cc * 128:(cc + 1) * 128, :])
            nc.scalar.dma_start(out=xT[cc + 1],
                                in_=xT_d[(cc + 1) * 128:(cc + 2) * 128, :])
        for cc in range(0, CCH, 2):
            nc.sync.dma_start(out=wv_sb[0][cc], in_=wv_d[0, cc])
            nc.scalar.dma_start(out=wv_sb[0][cc + 1], in_=wv_d[0, cc + 1])
        nc.sync.dma_start(out=wq8_sb[2], in_=wq8_d[2])
        nc.scalar.dma_start(out=wq8_sb[10], in_=wq8_d[10])
        nc.sync.dma_start(out=wq8_sb[3], in_=wq8_d[3])
        nc.scalar.dma_start(out=wq8_sb[11], in_=wq8_d[11])
        for cc in range(0, CCH, 2):
            nc.sync.dma_start(out=wv_sb[1][cc], in_=wv_d[1, cc])
            nc.scalar.dma_start(out=wv_sb[1][cc + 1], in_=wv_d[1, cc + 1])
        for jt in (4, 5, 6, 7):
            nc.sync.dma_start(out=wq8_sb[jt], in_=wq8_d[jt])
            nc.scalar.dma_start(out=wq8_sb[jt + 8], in_=wq8_d[jt + 8])
        nc.sync.dma_start(out=wo_sb[0], in_=wo_d[0])
        nc.scalar.dma_start(out=wo_sb[1], in_=wo_d[1])

        # denominator (ones) columns of vp, once.  Ones sit LAST within each
        # head's 65-wide block: PSUM partition access must be 32-aligned, so
        # the norm-mult must read po[0:64] (offset 0) and l lives at row 64.
        for ti in range(TCH):
            vcol = vp[ti].rearrange("p (h d) -> p h d", h=H)
            nc.vector.memset(vcol[:, :, D:D + 1], 1.0)

        # ---- building blocks ----
        def qk_group(jt, half):
            sl = slice(half * 512, (half + 1) * 512)
            ps = P1.tile([128, 512], FP32, tag="p1", name="psqk")
            for p in range(NPAIR):
                nc.tensor.matmul(
                    out=ps,
                    lhsT=wq8_sb[jt][:, p],
                    rhs=x8[p][:, :, sl],
                    start=(p == 0), stop=(p == NPAIR - 1),
                    perf_mode=DR)
            nc.vector.tensor_scalar(
                out=qkT[jt][:, sl], in0=ps,
                scalar1=1.0 / WS, scalar2=bqT[:, jt:jt + 1],
                op0=MULT, op1=ADD)

        def vproj(ti, jvt):
            vcol = vp[ti].rearrange("p (h d) -> p h d", h=H)
            ps = P1.tile([128, 512], FP32, tag="p1", name="psv")
            for cc in range(CCH):
                nc.tensor.matmul(
                    out=ps,
                    lhsT=xT[cc][:, ti * 128:(ti + 1) * 128],
                    rhs=wv_sb[jvt][cc],
                    start=(cc == 0), stop=(cc == CCH - 1))
            nc.vector.tensor_tensor(
                out=vcol[:, jvt * 8:(jvt + 1) * 8, 0:D],
                in0=ps.rearrange("p (h d) -> p h d", h=8),
                in1=bvb[:, jvt * 512:(jvt + 1) * 512].rearrange(
                    "p (h d) -> p h d", h=8),
                op=ADD)

        def outproj(ti, half):
            sl = slice(half * 512, (half + 1) * 512)
            ps = P1.tile([128, 512], FP32, tag="p1", name="pso")
            for hc in range(CCH):
                nc.tensor.matmul(
                    out=ps,
                    lhsT=oT[hc][:, ti * 128:(ti + 1) * 128],
                    rhs=wo_sb[half][:, hc, :],
                    start=(hc == 0), stop=(hc == CCH - 1))
            ot = outs.tile([128, 512], FP32, tag="ot", name="ot")
            nc.vector.tensor_tensor(out=ot, in0=ps, in1=bob[:, sl], op=ADD)
            nc.sync.dma_start(
                out=out_d[ti * 128:(ti + 1) * 128, sl], in_=ot)

        # ---- phase A (minimal: just what scores hp0 needs; everything else
        # streams in as phase-B filler) ----
        qk_group(0, 0)
        qk_group(8, 0)

        # ---- phase B: attention, it-outer, with PE filler interleave ----
        def make_attnv(hp, jc, njc, pT, s0, po2):
            def go():
                if jc == 0:
                    # allocate lazily at emission time so the ring-slot reuse
                    # dependency sees the previous pair's (lagged) norm reads
                    po2[0] = P3.tile([65, 512], FP32, tag="po0", name="po0")
                    po2[1] = P3.tile([65, 512], FP32, tag="po1", name="po1")
                for hx, h in enumerate((2 * hp, 2 * hp + 1)):
                    hsl = slice(h * (D + 1), h * (D + 1) + D + 1)
                    nc.tensor.matmul(
                        out=po2[hx][0:65, s0:512],
                        lhsT=vp[jc][:, hsl],
                        rhs=pT[:, hx, s0:512],
                        start=(jc == 0), stop=(jc == njc - 1),
                        skip_group_check=True)
            return go

        def make_norm(hp, it, po2):
            def go():
                # both hx chains interleaved so the gpsimd broadcast of hx0
                # overlaps the DVE copy/recip of hx1 (critical at flush tails)
                l_sb, rb1, rb = [None, None], [None, None], [None, None]
                for hx in range(2):
                    l_sb[hx] = lpool.tile([1, 512], FP32, tag="l", name="l")
                    # custom-DVE ops misread partition-offset inputs; extract
                    # the l row with a plain copy first
                    nc.vector.tensor_copy(out=l_sb[hx], in_=po2[hx][64:65, :])
                for hx in range(2):
                    rb1[hx] = lpool.tile([1, 512], FP32, tag="rb1", name="rb1")
                    nc.vector.reciprocal_approx_fast(out=rb1[hx], in_=l_sb[hx])
                    rb[hx] = rbpool.tile([64, 512], FP32, tag="rb", name="rb")
                    nc.gpsimd.partition_broadcast(out_ap=rb[hx], in_ap=rb1[hx])
                for hx in range(2):
                    nc.vector.tensor_tensor(
                        out=oT[hp][hx * 64:(hx + 1) * 64,
                                   it * 512:(it + 1) * 512],
                        in0=po2[hx][0:64, :], in1=rb[hx], op=MULT)
            return go

        # filler schedules: {unit_index: [(kind, args)]}
        # it0 (32 units): remaining qk halves in scores-consumption order +
        # all 16 vprojs (vp[jc] jvt0 needed by attnv hp<4; jvt1 by hp>=4)
        # NOTE: a filler must be EMITTED (unit index) before any lagged attnv
        # that reads its output pops from the pending queue — the tile
        # tracker orders by emission, so a late write is an untracked race.
        # attnv(hp0, jc) pops at unit jc+2 -> vproj(jc, 0) must sit at unit
        # <= jc+1; scores(hp, jc0) needs its qk halves strictly earlier.
        f_b1 = {}
        b1 = [(0, ("V", (0, 0))), (0, ("QK", (1, 0))),
              (1, ("V", (1, 0))), (1, ("QK", (9, 0))),
              (2, ("V", (2, 0))), (3, ("V", (3, 0))),
              (4, ("QK", (2, 0))), (5, ("QK", (10, 0))),
              (6, ("V", (0, 1))), (7, ("V", (1, 1))),
              (8, ("QK", (3, 0))), (9, ("QK", (11, 0))),
              (10, ("V", (2, 1))), (11, ("V", (3, 1))),
              (12, ("QK", (4, 0))), (13, ("QK", (12, 0))),
              (14, ("QK", (5, 0))), (15, ("QK", (13, 0))),
              (16, ("QK", (6, 0))), (17, ("QK", (14, 0))),
              (18, ("QK", (7, 0))), (19, ("QK", (15, 0))),
              (20, ("QK", (0, 1))), (21, ("QK", (8, 1))),
              (22, ("QK", (1, 1))), (23, ("QK", (9, 1)))]
        for u, g in b1:
            f_b1.setdefault(u, []).append(g)
        # it1 (64 units): vp[4..7], remaining qk h1 halves, out-proj t 0:512
        f_b2 = {}
        b2 = [("V", (4, 0)), ("V", (5, 0)), ("V", (6, 0)), ("V", (7, 0)),
              ("QK", (2, 1)), ("QK", (10, 1)), ("V", (4, 1)), ("V", (5, 1)),
              ("QK", (3, 1)), ("QK", (11, 1)), ("V", (6, 1)), ("V", (7, 1)),
              ("QK", (4, 1)), ("QK", (12, 1)), ("QK", (5, 1)), ("QK", (13, 1)),
              ("QK", (6, 1)), ("QK", (14, 1)), ("QK", (7, 1)), ("QK", (15, 1))]
        for u, g in enumerate(b2):
            f_b2.setdefault(u, []).append(g)
        b2o = [(0, 0), (0, 1), (1, 0), (1, 1), (2, 0), (2, 1), (3, 0), (3, 1)]
        for i, o in enumerate(b2o):
            f_b2.setdefault(24 + 3 * i, []).append(("OUT", o))

        # LAG=4: attnv pops 4 units after its scores so the early-it0 pops
        # land after the (DMA-bound) vproj fillers' inputs have arrived
        LAG = 4
        pending = deque()

        def flush(n_keep):
            while len(pending) > n_keep:
                pending.popleft()()

        for it in (0, 1):
            fill = f_b1 if it == 0 else f_b2
            njc = 4 * (it + 1)
            u = 0
            for hp in range(8):
                po2 = [None, None]
                for jc in range(njc):
                    s0 = max(0, jc * 128 - it * 512)
                    ps = P2.tile([128, 2, 512], FP32, tag="ps", name="ps")
                    pT = ppool.tile([128, 2, 512], BF16, tag="pT", name="pT")
                    for hx in range(2):
                        prow = slice(hx * 64, hx * 64 + 64)
                        nc.tensor.matmul(
                            out=ps[:, hx, s0:512],
                            lhsT=qkT[8 + hp][prow, jc * 128:(jc + 1) * 128],
                            rhs=qkT[hp][prow, it * 512 + s0:(it + 1) * 512],
                            start=True, stop=True)
                    nc.scalar.activation(
                        out=pT[:, :, s0:512], in_=ps[:, :, s0:512],
                        func=mybir.ActivationFunctionType.Exp, scale=0.125)
                    if jc >= it * 4:  # diagonal block: post-exp causal zeroing
                        nc.gpsimd.affine_select(
                            out=pT[:, :, s0:s0 + 128],
                            in_=pT[:, :, s0:s0 + 128],
                            compare_op=mybir.AluOpType.is_ge, fill=0.0,
                            base=0, channel_multiplier=-1,
                            pattern=[[0, 2], [1, 128]])
                    for kind, args in fill.get(u, ()):
                        if kind == "OUT":
                            outproj(*args)
                        elif kind == "V":
                            vproj(*args)
                        else:
                            qk_group(*args)
                    pending.append(make_attnv(hp, jc, njc, pT, s0, po2))
                    flush(LAG)
                    u += 1
                pending.append(make_norm(hp, it, po2))
            flush(0)

        # ---- phase C: out projection, token tiles 4..7 ----
        for ti in range(4, TCH):
            for half in range(2):
                outproj(ti, half)

    nc.compile()
    nc.finalize()
    return nc


_CACHE = {}


def _bf16(a):
    return np.ascontiguousarray(np.asarray(a, np.float32)).astype(
        ml_dtypes.bfloat16)


def _fp8(a):
    return np.ascontiguousarray(
        np.clip(np.asarray(a, np.float32), -240.0, 240.0)).astype(
        ml_dtypes.float8_e4m3)


def make_in_maps(x, W_qkv, b_qkv, W_out, b_out):
    W_qkv = np.asarray(W_qkv, np.float32)
    W_out = np.asarray(W_out, np.float32)
    b_qkv = np.asarray(b_qkv, np.float32).reshape(-1)
    b_out = np.asarray(b_out, np.float32).reshape(1, -1)
    x = np.asarray(x, np.float32)

    # fp8 q/k weights, 32x scaled: [16 jt][128 part, 4 pair, 2, 128 col]
    # element [part, p, e, col] = 32*Wqkv[128*(2p+e)+part, 128*jt+col]
    wq8 = np.stack([
        _fp8((WS * W_qkv[:, jt * 128:(jt + 1) * 128])
             .reshape(NPAIR, 2, 128, 128).transpose(2, 0, 1, 3))
        for jt in range(16)])
    wv = np.stack([
        _bf16(W_qkv[:, 2 * C + j * 512:2 * C + (j + 1) * 512]
              .reshape(CCH, 128, 512))
        for j in range(2)])
    wo = np.stack([
        _bf16(W_out[:, h * 512:(h + 1) * 512]
              .reshape(CCH, 128, 512).transpose(1, 0, 2))
        for h in range(2)])
    bqT = np.ascontiguousarray(b_qkv[:2 * C].reshape(16, 128).T)
    bqv = np.ascontiguousarray(b_qkv[2 * C:].reshape(1, C))

    shared = {
        "wq8": wq8, "wv": wv, "wo": wo,
        "bqT": bqT, "bqv": bqv, "bo": np.ascontiguousarray(b_out),
    }
    maps = []
    for i in range(N_CORES):
        xT = x[i].T  # [C, T]
        # x8 pairs: [4, 128, 2, T]: [p, part, e, t] = x[t, 128*(2p+e)+part]
        x8 = _fp8(xT.reshape(NPAIR, 2, 128, T).transpose(0, 2, 1, 3))
        maps.append(dict(shared, x8_b=x8, xT_b=_bf16(xT)))
    return maps


def kernel(x, W_qkv, b_qkv, W_out, b_out):
    if "nc" not in _CACHE:
        _CACHE["nc"] = build_nc()
    nc = _CACHE["nc"]
    in_maps = make_in_maps(x, W_qkv, b_qkv, W_out, b_out)
    res = bass_utils.run_bass_kernel_spmd(nc, in_maps, core_ids=list(range(N_CORES)))
    return np.stack([r["out_b"] for r in res.results]).astype(np.float32)
